# revision 1
# baseline (speedup 1.0000x reference)
"""DiffAttention Trainium2 Bass kernel (8-core head-parallel SPMD).

Contract: kernel(**inputs) takes the FULL inputs from setup_inputs() and
returns the FULL (B, S, DIM) output. Internally it shards the 16 heads
across 8 NeuronCores (2 heads/core); each core is fully independent (the
reference's transpose-then-reshape makes each head own a contiguous block
of 256 output rows, so no collectives are needed).

Per-core dataflow (all fp32):
  phase A: PE-transpose wq/wk/wv head-slices to feature-major layout
  phase B: PE-transpose x tiles on the fly; Q^T/K^T (feature-major) and V
           (token-major) projections; spill raw Q^T/K^T/V to DRAM scratch
  phase D: per (batch, head): reload Q^T/K^T straight + partition-swapped,
           RoPE as 3 lane-aligned DVE ops with host-baked cos/sin tables;
           flash-style two-stream attention on transposed scores
           (keys on partitions) with PE ones-matmul row sums, diff-combine,
           folded RMSNorm; accumulate feature-major attnN per (b,h)
  phase C: PE-transpose wo blocks on the fly; final projection with the
           "scrambled reshape" expressed as a stride-16 stationary AP
"""

import numpy as np
from contextlib import ExitStack

import concourse.bass as bass
import concourse.bacc as bacc
import concourse.tile as tile
from concourse import mybir
from concourse.masks import make_identity
from concourse.bass_utils import run_bass_kernel_spmd

F32 = mybir.dt.float32
F32R = mybir.dt.float32r
AF = mybir.ActivationFunctionType
OP = mybir.AluOpType

B, S, DIM = 2, 2048, 2048
NH, HD, HHD = 16, 128, 64
NC = 8
HPC = NH // NC          # 2 heads per core
E = HPC * HD            # 256 projection rows per core
T = B * S               # 4096 flattened tokens
ND = DIM // 128         # 16 d-tiles
NTB = T // 512          # 8 t-blocks
LAMBDA_INIT = 0.2
EPS = 1e-5


def _mmr(nc, out, lhsT, rhs, **kw):
    """matmul with float32r operand views (4x faster than fp32 on the PE)."""
    nc.tensor.matmul(out, lhsT.bitcast(F32R), rhs.bitcast(F32R), **kw)


def _tpose4(nc, pool, evict, dst4, stg, ident, cols, tag):
    """Four 128x128 PE transposes into one PSUM bank + one fat eviction.

    dst4: strided destination AP [128, 4, 128]; stg: source tile;
    cols: iterable of 4 column offsets in stg. evict: copy fn(out, in_).
    """
    tp = pool.tile([128, 512], F32, tag=tag, name=tag)
    for i, c0 in enumerate(cols):
        nc.tensor.matmul(tp[:, i * 128:(i + 1) * 128].bitcast(F32R),
                         stg[:, c0:c0 + 128].bitcast(F32R),
                         ident.bitcast(F32R),
                         is_transpose=True, skip_group_check=True)
    evict(out=dst4.bitcast(F32R), in_=tp)


_CACHE = {}


def _build_program(nrep=1):
    nc = bacc.Bacc("TRN2", target_bir_lowering=False, debug=False, num_devices=NC)

    x_d = nc.dram_tensor("x", [T, DIM], F32R, kind="ExternalInput").ap()
    wq_d = nc.dram_tensor("wq", [E, DIM], F32R, kind="ExternalInput").ap()
    wk_d = nc.dram_tensor("wk", [E, DIM], F32R, kind="ExternalInput").ap()
    wv_d = nc.dram_tensor("wv", [E, DIM], F32R, kind="ExternalInput").ap()
    wo_d = nc.dram_tensor("wo", [DIM, DIM], F32R, kind="ExternalInput").ap()
    cosq_d = nc.dram_tensor("cosq", [128, S], F32, kind="ExternalInput").ap()
    sinq_d = nc.dram_tensor("sinq", [128, S], F32, kind="ExternalInput").ap()
    cosk_d = nc.dram_tensor("cosk", [128, S], F32, kind="ExternalInput").ap()
    sink_d = nc.dram_tensor("sink", [128, S], F32, kind="ExternalInput").ap()
    ident_d = nc.dram_tensor("ident", [128, 128], F32R, kind="ExternalInput").ap()
    mask_d = nc.dram_tensor("mask", [128, 896], F32, kind="ExternalInput").ap()
    subw_d = nc.dram_tensor("subw", [128, 1], F32, kind="ExternalInput").ap()
    lam_d = nc.dram_tensor("lam", [1, 1], F32, kind="ExternalInput").ap()
    out_d = nc.dram_tensor("out", [B, E, DIM], F32, kind="ExternalOutput").ap()

    # DRAM scratch for raw (unroped) projections, feature-major / token-major
    qT_d = {(h, b): nc.dram_tensor(f"qTs{h}_{b}", [128, S], F32).ap()
            for h in range(HPC) for b in range(B)}
    kT_d = {(h, b): nc.dram_tensor(f"kTs{h}_{b}", [128, S], F32).ap()
            for h in range(HPC) for b in range(B)}
    v2_d = {b: nc.dram_tensor(f"vs2_{b}", [128, HPC * S], F32).ap()
            for b in range(B)}
    v2v = {b: v2_d[b].rearrange("p (h tt u) -> p h tt u", h=HPC, tt=S // 128)
           for b in range(B)}

    with tile.TileContext(nc) as tc:
        for rep in range(nrep):
            ctx = ExitStack()
            consts = ctx.enter_context(tc.tile_pool(name="consts", bufs=1))
            ident = consts.tile([128, 128], F32)
            nc.sync.dma_start(out=ident.bitcast(F32R), in_=ident_d)
            mask_t = consts.tile([128, 896], F32)
            nc.sync.dma_start(out=mask_t, in_=mask_d)
            ones_tmp = consts.tile([128, 1], F32)
            nc.vector.memset(ones_tmp, 1.0)
            ones_col = consts.tile([128, 1], F32)
            nc.scalar.copy(out=ones_col.bitcast(F32R), in_=ones_tmp)
            subw_t = consts.tile([128, 1], F32)
            nc.sync.dma_start(out=subw_t, in_=subw_d)
            lam_t = consts.tile([1, 1], F32)
            nc.sync.dma_start(out=lam_t, in_=lam_d)

            # ---------------- phase A: weight transposes ----------------
            # wT_all free layout: dt*E + et*128 + u
            wT = {}
            wTv = {}
            ab_ctx = ExitStack()
            wTpool = ab_ctx.enter_context(tc.tile_pool(name="wT", bufs=1))
            with tc.tile_pool(name="wstage", bufs=3) as wstage, \
                 tc.tile_pool(name="psA", bufs=4, space="PSUM") as psA:
                for wname, wd in (("q", wq_d), ("k", wk_d), ("v", wv_d)):
                    wall = wTpool.tile([128, ND * E], F32, tag=f"w{wname}T",
                                       name=f"w{wname}T")
                    wT[wname] = wall
                    wTv[wname] = wall[:].rearrange(
                        "p (d e u) -> p d e u", d=ND, e=HPC)
                    for et in range(HPC):
                        stg = wstage.tile([128, DIM], F32, tag="wstg", name="wstg")
                        nc.sync.dma_start(out=stg.bitcast(F32R), in_=wd[et * 128:(et + 1) * 128, :])
                        for dtg in range(0, ND, 4):
                            _tpose4(nc, psA, nc.scalar.copy,
                                    wTv[wname][:, dtg:dtg + 4, et, :], stg, ident,
                                    [(dtg + i) * 128 for i in range(4)], "tpA")

            d_ctx = ExitStack()
            qkv = d_ctx.enter_context(tc.tile_pool(name="qkv", bufs=1, side="right"))

            # ---------------- phase B: x transposes + projections ----------------
            with tc.tile_pool(name="xstage", bufs=4) as xstage, \
                 tc.tile_pool(name="xT", bufs=1) as xTpool, \
                 tc.tile_pool(name="psT", bufs=4, space="PSUM") as psT, \
                 tc.tile_pool(name="psP", bufs=2, space="PSUM") as psP, \
                 tc.tile_pool(name="sstage", bufs=4) as sstage:
                for tb in range(NTB):
                    # xT_all free layout: dt*512 + ts*128 + u
                    xT = xTpool.tile([128, ND * 512], F32, tag="xTa", name="xTa")
                    xTv = xT[:].rearrange("p (d t u) -> p d t u", d=ND, t=4)
                    for ts in range(4):
                        xs = xstage.tile([128, DIM], F32, tag="xs", name="xs")
                        r0 = tb * 512 + ts * 128
                        nc.sync.dma_start(out=xs.bitcast(F32R), in_=x_d[r0:r0 + 128, :])
                        for dtg in range(0, ND, 4):
                            _tpose4(nc, psT, nc.vector.tensor_copy,
                                    xTv[:, dtg:dtg + 4, ts, :], xs, ident,
                                    [(dtg + i) * 128 for i in range(4)], "tpB")
                    # Q^T, K^T feature-major: out[e,t] = sum_d wT[d,e] * xT[d,t]
                    for wname, dst in (("q", qT_d), ("k", kT_d)):
                        for et in range(HPC):
                            pp = psP.tile([128, 512], F32, tag="qkp", name="qkp")
                            for dt in range(ND):
                                _mmr(nc, pp, wTv[wname][:, dt, et, :],
                                     xT[:, dt * 512:(dt + 1) * 512],
                                     start=(dt == 0), stop=(dt == ND - 1))
                            st = sstage.tile([128, 512], F32, tag="qks", name="qks")
                            nc.scalar.copy(out=st.bitcast(F32R), in_=pp)
                            bb, trel = divmod(tb, 4)
                            nc.sync.dma_start(
                                out=dst[(et, bb)][:, trel * 512:(trel + 1) * 512],
                                in_=st)
                    # V token-major: out[t,hd] = sum_d xT[d,t] * wvT[d,hd]
                    for ts in range(4):
                        pp = psP.tile([128, E], F32, tag="vp", name="vp")
                        for dt in range(ND):
                            _mmr(nc, pp, xTv[:, dt, ts, :],
                                 wT["v"][:, dt * E:(dt + 1) * E],
                                 start=(dt == 0), stop=(dt == ND - 1))
                        sv = sstage.tile([128, E], F32, tag="vs", name="vs")
                        nc.scalar.copy(out=sv.bitcast(F32R), in_=pp)
                        bb = tb // 4
                        tt = (tb % 4) * 4 + ts
                        nc.sync.dma_start(
                            out=v2v[bb][:, :, tt, :], in_=sv[:].rearrange(
                                "p (h u) -> p h u", h=HPC))
            ab_ctx.close()

            # ---------------- phase D: attention per (b, head) ----------------
            attnN_pool = ctx.enter_context(tc.tile_pool(name="attnN", bufs=1))
            attnN = {}
            msbuf = {}
            cs = {}
            with tc.tile_pool(name="ropec", bufs=1) as ropec:
                for nm, d in (("cosq", cosq_d), ("sinq", sinq_d),
                              ("cosk", cosk_d), ("sink", sink_d)):
                    t_ = ropec.tile([128, S], F32, tag=nm, name=nm)
                    nc.sync.dma_start(out=t_, in_=d)
                    cs[nm] = t_

                with tc.tile_pool(name="ropes", bufs=1) as ropes, \
                     tc.tile_pool(name="expp", bufs=3) as expp, \
                     tc.tile_pool(name="cmb", bufs=1) as cmb, \
                     tc.tile_pool(name="psS", bufs=3, space="PSUM") as psS, \
                     tc.tile_pool(name="psAU", bufs=1, space="PSUM") as psAU, \
                     tc.tile_pool(name="psL", bufs=1, space="PSUM") as psL:
                    for b in range(B):
                        for hl in range(HPC):
                            at_t = attnN_pool.tile([128, S], F32, tag=f"attnN{b}_{hl}", name=f"attnN{b}_{hl}")
                            attnN[(b, hl)] = at_t
                            msb = attnN_pool.tile([1, S], F32, tag=f"msb{b}_{hl}", name=f"msb{b}_{hl}")
                            msbuf[(b, hl)] = msb
                            qr = qkv.tile([128, S], F32, tag="qr", name="qr", bufs=2)
                            kr = qkv.tile([128, S], F32, tag="kr", name="kr", bufs=2)
                            vh = qkv.tile([128, S], F32, tag="vh", name="vh", bufs=2)
                            qsw = qkv.tile([128, S], F32, tag="qsw", name="qsw")
                            ksw = qkv.tile([128, S], F32, tag="ksw", name="ksw")
                            nc.sync.dma_start(out=qr, in_=qT_d[(hl, b)])
                            nc.sync.dma_start(out=kr, in_=kT_d[(hl, b)])
                            nc.sync.dma_start(
                                out=vh, in_=v2_d[b][:, hl * S:(hl + 1) * S])
                            # partition-swapped copies: rows [32:64,0:32,96:128,64:96]
                            for dst, src in ((qsw, qT_d[(hl, b)]),
                                             (ksw, kT_d[(hl, b)])):
                                for blk in range(4):
                                    sb = (blk ^ 1) * 32
                                    nc.sync.dma_start(
                                        out=dst[blk * 32:(blk + 1) * 32, :],
                                        in_=src[sb:sb + 32, :])
                            # rope: t = t*cos + tsw*sinsgn (q pre-scaled by 1/8)
                            for t_, sw_, cn, sn in ((qr, qsw, "cosq", "sinq"),
                                                    (kr, ksw, "cosk", "sink")):
                                m1 = ropes.tile([128, S], F32, tag="m1", name="m1")
                                nc.vector.tensor_mul(m1, t_, cs[cn])
                                nc.vector.tensor_mul(sw_, sw_, cs[sn])
                                nc.vector.tensor_add(t_.bitcast(F32R), m1, sw_)

                            for ib in range(4):
                                i0 = ib * 512
                                njt = ib * 4 + 4
                                au = [psAU.tile([128, 512], F32, tag=f"au{s_}", name=f"au{s_}")
                                      for s_ in range(2)]
                                Lap = [psL.tile([1, 512], F32, tag=f"L{s_}",
                                                name=f"L{s_}")[:]
                                       for s_ in range(2)]
                                for jt in range(njt):
                                    j0 = jt * 128
                                    r = jt - ib * 4
                                    c0 = max(r, 0) * 128  # cols left of this are
                                    # fully masked for diagonal tiles: skip them
                                    for s_ in range(2):
                                        e0 = s_ * 64
                                        sp = psS.tile([128, 512], F32,
                                                      tag="sp", name="sp", bufs=3)
                                        _mmr(nc, sp[:, c0:512],
                                             kr[e0:e0 + 64, j0:j0 + 128],
                                             qr[e0:e0 + 64, i0 + c0:i0 + 512])
                                        ex = expp.tile([128, 512], F32,
                                                       tag=f"ex{s_}", name=f"ex{s_}")
                                        nc.scalar.activation(
                                            out=ex[:, c0:512].bitcast(F32R),
                                            in_=sp[:, c0:512], func=AF.Exp)
                                        if r >= 0:
                                            m0 = r * 128  # triangle block
                                            nc.vector.tensor_mul(
                                                ex[:, m0:m0 + 128].bitcast(F32R),
                                                ex[:, m0:m0 + 128],
                                                mask_t[:, 384:512])
                                        _mmr(nc, au[s_][:, c0:512],
                                             vh[:, j0:j0 + 128], ex[:, c0:512],
                                             start=(jt == 0), stop=(jt == njt - 1))
                                        _mmr(nc, Lap[s_][:, c0:512], ones_col,
                                             ex[:, c0:512],
                                             start=(jt == 0), stop=(jt == njt - 1),
                                             skip_group_check=True)
                                # combine: au0/L0 - lam * au1/L1 (RMSNorm deferred)
                                r1 = cmb.tile([1, 512], F32, tag="r1", name="r1")
                                r2 = cmb.tile([1, 512], F32, tag="r2", name="r2")
                                nc.vector.reciprocal(r1, Lap[0])
                                nc.vector.reciprocal(r2, Lap[1])
                                nc.vector.tensor_scalar_mul(r2, r2, lam_t[:])
                                br1 = cmb.tile([128, 512], F32, tag="br1", name="br1")
                                br2 = cmb.tile([128, 512], F32, tag="br2", name="br2")
                                nc.gpsimd.partition_broadcast(br1, r1)
                                nc.gpsimd.partition_broadcast(br2, r2)
                                t1 = cmb.tile([128, 512], F32, tag="t1", name="t1")
                                t2 = cmb.tile([128, 512], F32, tag="t2", name="t2")
                                nc.vector.tensor_mul(t1, au[0], br1)
                                nc.vector.tensor_mul(t2, au[1], br2)
                                at = at_t[:, i0:i0 + 512]
                                nc.vector.tensor_sub(at.bitcast(F32R), t1, t2)
                                sq = cmb.tile([128, 512], F32, tag="sq", name="sq")
                                nc.vector.tensor_mul(sq.bitcast(F32R), at, at)
                                msp = psL.tile([1, 512], F32, tag="msp", name="msp")
                                _mmr(nc, msp, ones_col, sq)
                                # msbuf <- ms/128 + eps (one fused DVE op)
                                nc.vector.tensor_scalar(
                                    msb[:, i0:i0 + 512], msp, 1.0 / 128.0, EPS,
                                    OP.mult, OP.add)

            d_ctx.close()

            # ------------- phase D': deferred RMSNorm (batched sqrt) -------------
            with tc.tile_pool(name="dnorm", bufs=3) as dnorm:
                for b in range(B):
                    for hl in range(HPC):
                        at_t = attnN[(b, hl)]
                        msb = msbuf[(b, hl)]
                        rinv = dnorm.tile([1, S], F32, tag="rinv", name="rinv")
                        nc.vector.reciprocal(rinv, msb)
                        rs = dnorm.tile([1, S], F32, tag="rs", name="rs")
                        nc.scalar.activation(out=rs, in_=rinv, func=AF.Sqrt)
                        brs = dnorm.tile([128, S], F32, tag="brs", name="brs")
                        nc.gpsimd.partition_broadcast(brs, rs)
                        tn = dnorm.tile([128, S], F32, tag="tn", name="tn")
                        nc.vector.tensor_mul(tn, at_t, brs)
                        nc.scalar.activation(
                            out=at_t[:].bitcast(F32R), in_=tn,
                            func=AF.Copy, scale=subw_t[:])

            # ---------------- phase C: output projection ----------------
            with tc.tile_pool(name="wostage", bufs=2) as wostage, \
                 tc.tile_pool(name="woT", bufs=1) as woTpool, \
                 tc.tile_pool(name="psC", bufs=4, space="PSUM") as psC, \
                 tc.tile_pool(name="psO", bufs=2, space="PSUM") as psO, \
                 tc.tile_pool(name="ostage", bufs=4) as ostage:
                for mb in range(4):
                    # woT_all free layout: v16*512 + j4*128 + u
                    woT = woTpool.tile([128, 16 * 512], F32, tag="woTa", name="woTa")
                    woTv = woT[:].rearrange("p (v t u) -> p v t u", v=16, t=4)
                    for j4 in range(4):
                        ws = wostage.tile([128, DIM], F32, tag="wos", name="wos")
                        r0 = mb * 512 + j4 * 128
                        nc.sync.dma_start(out=ws.bitcast(F32R), in_=wo_d[r0:r0 + 128, :])
                        for vg in range(0, 16, 4):
                            _tpose4(nc, psC, nc.scalar.copy,
                                    woTv[:, vg:vg + 4, j4, :], ws, ident,
                                    [(vg + i) * 128 for i in range(4)], "tpC")
                    for b in range(B):
                        for hl in range(HPC):
                            at_t = attnN[(b, hl)]
                            y = at_t[:].rearrange("p (u v) -> p v u", v=16)
                            op = psO.tile([128, 512], F32, tag="op", name="op")
                            for v16 in range(16):
                                _mmr(nc, op, y[:, v16, :],
                                     woT[:, v16 * 512:(v16 + 1) * 512],
                                     start=(v16 == 0), stop=(v16 == 15))
                            ost = ostage.tile([128, 512], F32, tag="ost", name="ost")
                            nc.scalar.copy(out=ost, in_=op)
                            nc.sync.dma_start(
                                out=out_d[b, hl * 128:(hl + 1) * 128,
                                          mb * 512:(mb + 1) * 512],
                                in_=ost)

            ctx.close()

    nc.compile()
    return nc


def get_program(nrep=1):
    key = f"nc{nrep}"
    if key not in _CACHE:
        _CACHE[key] = _build_program(nrep)
    return _CACHE[key]


def _prep_in_maps(inputs):
    inp = {k: np.ascontiguousarray(np.asarray(v, dtype=np.float32))
           for k, v in inputs.items()}
    perm = np.concatenate([
        np.arange(0, 64, 2), np.arange(1, 64, 2),
        np.arange(64, 128, 2), np.arange(65, 128, 2)])
    wq_p = inp["wq"].reshape(NH, HD, DIM)[:, perm, :].reshape(NH * HD, DIM)
    wk_p = inp["wk"].reshape(NH, HD, DIM)[:, perm, :].reshape(NH * HD, DIM)

    fc = inp["freq_cis"]
    cosP = fc[:, :, 0, 0].T.astype(np.float32)
    sinP = fc[:, :, 1, 0].T.astype(np.float32)
    COS = np.concatenate([cosP[0:32], cosP[0:32], cosP[32:64], cosP[32:64]], 0)
    SIN = np.concatenate([-sinP[0:32], sinP[0:32], -sinP[32:64], sinP[32:64]], 0)

    ident = np.eye(128, dtype=np.float32)
    mask = (np.arange(128)[:, None] <= np.arange(896)[None, :] - 384)
    mask = np.ascontiguousarray(mask.astype(np.float32))

    lam1 = np.exp(np.sum(inp["lambda_q1"] * inp["lambda_k1"], dtype=np.float32))
    lam2 = np.exp(np.sum(inp["lambda_q2"] * inp["lambda_k2"], dtype=np.float32))
    lam = np.array([[lam1 - lam2 + LAMBDA_INIT]], np.float32)
    subw = np.ascontiguousarray(
        (inp["subln_w"] * (1.0 - LAMBDA_INIT)).astype(np.float32).reshape(128, 1))

    x_f = np.ascontiguousarray(inp["x"].reshape(T, DIM))
    common = {
        "x": x_f, "wo": inp["wo"],
        "cosq": np.ascontiguousarray(COS * 0.125),
        "sinq": np.ascontiguousarray(SIN * 0.125),
        "cosk": np.ascontiguousarray(COS),
        "sink": np.ascontiguousarray(SIN),
        "ident": ident, "mask": mask,
        "subw": subw, "lam": lam,
    }
    in_maps = []
    for c in range(NC):
        m = dict(common)
        m["wq"] = np.ascontiguousarray(wq_p[c * E:(c + 1) * E])
        m["wk"] = np.ascontiguousarray(wk_p[c * E:(c + 1) * E])
        m["wv"] = np.ascontiguousarray(inp["wv"][c * E:(c + 1) * E])
        in_maps.append(m)
    return in_maps


def run(inputs, trace=False, **kw):
    nc = get_program()
    in_maps = _prep_in_maps(inputs)
    res = run_bass_kernel_spmd(nc, in_maps, list(range(NC)), trace=trace, **kw)
    out = np.empty((B, S, DIM), np.float32)
    for c in range(NC):
        out[:, c * E:(c + 1) * E, :] = res.results[c]["out"]
    return out, res


def kernel(**inputs):
    out, _ = run(inputs)
    return out


# ---------------------------------------------------------------------------
# benchmarking helpers (wall-clock with device-resident inputs, null-calibrated)
# ---------------------------------------------------------------------------

def _make_sharded_callable(nc, in_maps, n_cores):
    import jax
    from jax.experimental.shard_map import shard_map
    from jax.sharding import Mesh, PartitionSpec, NamedSharding
    from concourse import bass2jax

    bass2jax.install_neuronx_cc_hook()
    partition_name = nc.partition_id_tensor.name if nc.partition_id_tensor else None
    in_names, out_names, out_avals, zero_outs = [], [], [], []
    for alloc in nc.m.functions[0].allocations:
        if not isinstance(alloc, mybir.MemoryLocationSet):
            continue
        name = alloc.memorylocations[0].name
        if alloc.kind == "ExternalInput":
            if name != partition_name:
                in_names.append(name)
        elif alloc.kind == "ExternalOutput":
            out_names.append(name)
            shape = tuple(alloc.tensor_shape)
            dtype = mybir.dt.np(alloc.dtype)
            out_avals.append(jax.core.ShapedArray(shape, dtype))
            zero_outs.append(np.zeros(shape, dtype))
    n_params = len(in_names)
    all_in = list(in_names) + list(out_names)
    if partition_name is not None:
        all_in.append(partition_name)

    def _body(*args):
        operands = list(args)
        if partition_name is not None:
            operands.append(bass2jax.partition_id_tensor())
        outs = bass2jax._bass_exec_p.bind(
            *operands,
            out_avals=tuple(out_avals),
            in_names=tuple(all_in),
            out_names=tuple(out_names),
            lowering_input_output_aliases=(),
            sim_require_finite=True,
            sim_require_nnan=True,
            nc=nc,
        )
        return tuple(outs)

    devices = jax.devices()[:n_cores]
    mesh = Mesh(np.asarray(devices), ("core",))
    in_specs = (PartitionSpec("core"),) * (n_params + len(out_names))
    out_specs = (PartitionSpec("core"),) * len(out_names)
    fn = jax.jit(shard_map(_body, mesh=mesh, in_specs=in_specs,
                           out_specs=out_specs, check_rep=False),
                 keep_unused=True)
    sh = NamedSharding(mesh, PartitionSpec("core"))
    per_core = [[np.asarray(m[n]) for n in in_names] for m in in_maps]
    args = [np.concatenate([per_core[c][i] for c in range(n_cores)], axis=0)
            for i in range(n_params)]
    args += [np.zeros((n_cores * z.shape[0], *z.shape[1:]), z.dtype)
             for z in zero_outs]
    dev_args = [jax.device_put(a, sh) for a in args]
    return fn, dev_args


def _time_calls(fn, dev_args, iters=8):
    import time as _t
    import jax
    out = fn(*dev_args)
    jax.block_until_ready(out)
    times = []
    for _ in range(iters):
        t0 = _t.perf_counter()
        out = fn(*dev_args)
        jax.block_until_ready(out)
        times.append(_t.perf_counter() - t0)
    return min(times), times


def _build_null_program():
    nc = bacc.Bacc("TRN2", target_bir_lowering=False, debug=False, num_devices=NC)
    o = nc.dram_tensor("nout", [1, 16], F32, kind="ExternalOutput").ap()
    with tile.TileContext(nc) as tc:
        with tc.tile_pool(name="p", bufs=1) as p:
            t_ = p.tile([1, 16], F32)
            nc.vector.memset(t_, 1.0)
            nc.sync.dma_start(out=o, in_=t_)
    nc.compile()
    return nc


def bench(inputs, iters=8):
    """Returns (kernel_min_s, null_min_s, est_exec_ns)."""
    nc = get_program()
    in_maps = _prep_in_maps(inputs)
    fn, dev_args = _make_sharded_callable(nc, in_maps, NC)
    tk, tk_all = _time_calls(fn, dev_args, iters)
    ncn = _build_null_program()
    fn0, dev0 = _make_sharded_callable(ncn, [{} for _ in range(NC)], NC)
    t0_, t0_all = _time_calls(fn0, dev0, iters)
    return tk, t0_, (tk - t0_) * 1e9, tk_all, t0_all


def _make_chained_callable(nc, in_maps, n_cores, nrep):
    import jax
    import jax.numpy as jnp
    from jax.experimental.shard_map import shard_map
    from jax.sharding import Mesh, PartitionSpec, NamedSharding
    from concourse import bass2jax

    bass2jax.install_neuronx_cc_hook()
    partition_name = nc.partition_id_tensor.name if nc.partition_id_tensor else None
    in_names, out_names, out_avals, zero_outs = [], [], [], []
    for alloc in nc.m.functions[0].allocations:
        if not isinstance(alloc, mybir.MemoryLocationSet):
            continue
        name = alloc.memorylocations[0].name
        if alloc.kind == "ExternalInput":
            if name != partition_name:
                in_names.append(name)
        elif alloc.kind == "ExternalOutput":
            out_names.append(name)
            shape = tuple(alloc.tensor_shape)
            dtype = mybir.dt.np(alloc.dtype)
            out_avals.append(jax.core.ShapedArray(shape, dtype))
            zero_outs.append(np.zeros(shape, dtype))
    n_params = len(in_names)
    il = in_names.index("lam")
    all_in = list(in_names) + list(out_names)
    if partition_name is not None:
        all_in.append(partition_name)

    def _bodyN(*args):
        ops = list(args)
        outs = None
        for _ in range(nrep):
            call = list(ops)
            if partition_name is not None:
                call.append(bass2jax.partition_id_tensor())
            outs = bass2jax._bass_exec_p.bind(
                *call,
                out_avals=tuple(out_avals),
                in_names=tuple(all_in),
                out_names=tuple(out_names),
                lowering_input_output_aliases=(),
                sim_require_finite=True,
                sim_require_nnan=True,
                nc=nc,
            )
            # serialize iterations: thread a zero-valued dep through lam
            ops[il] = ops[il] + outs[0].reshape(-1)[0] * 0.0
        return tuple(outs)

    devices = jax.devices()[:n_cores]
    mesh = Mesh(np.asarray(devices), ("core",))
    in_specs = (PartitionSpec("core"),) * (n_params + len(out_names))
    out_specs = (PartitionSpec("core"),) * len(out_names)
    fn = jax.jit(shard_map(_bodyN, mesh=mesh, in_specs=in_specs,
                           out_specs=out_specs, check_rep=False),
                 keep_unused=True)
    sh = NamedSharding(mesh, PartitionSpec("core"))
    per_core = [[np.asarray(m[n]) for n in in_names] for m in in_maps]
    args = [np.concatenate([per_core[c][i] for c in range(n_cores)], axis=0)
            for i in range(n_params)]
    args += [np.zeros((n_cores * z.shape[0], *z.shape[1:]), z.dtype)
             for z in zero_outs]
    dev_args = [jax.device_put(a, sh) for a in args]
    return fn, dev_args


def bench_chain(inputs, nrep=8, iters=5):
    nc = get_program()
    in_maps = _prep_in_maps(inputs)
    fn1, dev1 = _make_chained_callable(nc, in_maps, NC, 1)
    t1, t1_all = _time_calls(fn1, dev1, iters)
    fnN, devN = _make_chained_callable(nc, in_maps, NC, nrep)
    tN, tN_all = _time_calls(fnN, devN, iters)
    per = (tN - t1) / (nrep - 1)
    return per, t1, tN, t1_all, tN_all


def bench_rep(inputs, nrep=3, iters=20):
    """(T_nrep - T_1)/(nrep-1) from min wall times; dispatch cancels."""
    in_maps = _prep_in_maps(inputs)
    nc1 = get_program(1)
    fn1, dev1 = _make_sharded_callable(nc1, in_maps, NC)
    t1, t1_all = _time_calls(fn1, dev1, iters)
    ncN = get_program(nrep)
    fnN, devN = _make_sharded_callable(ncN, in_maps, NC)
    tN, tN_all = _time_calls(fnN, devN, iters)
    per = (tN - t1) / (nrep - 1)
    return per, t1, tN, t1_all, tN_all



# revision 2
# speedup vs baseline: 1.9692x; 1.9692x over previous
"""DiffAttention Trainium2 Bass kernel (8-core head-parallel SPMD), v2.

Contract: kernel(**inputs) takes the FULL inputs from setup_inputs() and
returns the FULL (B, S, DIM) output. Internally it shards the 16 heads
across 8 NeuronCores (2 heads/core); each core is fully independent (the
reference's transpose-then-reshape makes each head own a contiguous block
of 256 output rows, so no collectives are needed).

v2 design (vs the DRAM-spill baseline):
  - host pre-transposes x, wq/wk/wv (with the RoPE even/odd row perm) and
    wo, so the device does ZERO PE transposes; subln_w and (1-lambda_init)
    are folded into wo on the host; the 1/sqrt(64) score scale is folded
    into the Q PSUM eviction.
  - phase B computes Q^T/K^T (feature-major) and V (token-major) straight
    into resident SBUF tiles (bf16) -- no DRAM spill/reload.
  - RoPE per (head, batch) right after its 4 token-blocks finish: SBUF->
    SBUF partition-swap DMA + 3 DVE ops against host-baked cos/sin tables.
  - flash-style two-stream attention on transposed scores (keys on
    partitions): fp32r scores matmul, scalar-engine exp -> bf16, bf16 PV
    and ones-row-sum matmuls; diagonal tiles clamp the column start to 256
    so every matmul keeps the fp32r full-rate (>=256 moving) shape.
  - mean-square for the RMSNorm via gpsimd partition_all_reduce (keeps all
    8 PSUM banks for scores/PV/L); deferred norm uses DVE
    reciprocal_approx_fast + batched scalar Sqrt (avoids act-table thrash).
  - output projection with bf16 wo^T tiles, "scrambled reshape" expressed
    as a stride-16 stationary AP.
"""

import numpy as np
from contextlib import ExitStack

import ml_dtypes

import concourse.bass as bass
import concourse.bacc as bacc
import concourse.tile as tile
from concourse import mybir, bass_isa
from concourse.bass_utils import run_bass_kernel_spmd

F32 = mybir.dt.float32
F32R = mybir.dt.float32r
BF16 = mybir.dt.bfloat16
AF = mybir.ActivationFunctionType
OP = mybir.AluOpType

B, S, DIM = 2, 2048, 2048
NH, HD, HHD = 16, 128, 64
NC = 8
HPC = NH // NC          # 2 heads per core
E = HPC * HD            # 256 projection rows per core
T = B * S               # 4096 flattened tokens
ND = DIM // 128         # 16 d-tiles
NTB = T // 512          # 8 t-blocks
LAMBDA_INIT = 0.2
EPS = 1e-5

BF = ml_dtypes.bfloat16


def _mmr(nc, out, lhsT, rhs, **kw):
    """matmul with float32r operand views (full PE rate at >=256 moving)."""
    nc.tensor.matmul(out, lhsT.bitcast(F32R), rhs.bitcast(F32R), **kw)


_CACHE = {}


def _build_program(nrep=1):
    nc = bacc.Bacc("TRN2", target_bir_lowering=False, debug=False, num_devices=NC)

    xT_d = nc.dram_tensor("xT", [DIM, T], F32R, kind="ExternalInput").ap()
    wqT_d = nc.dram_tensor("wqT", [DIM, E], F32R, kind="ExternalInput").ap()
    wkT_d = nc.dram_tensor("wkT", [DIM, E], F32R, kind="ExternalInput").ap()
    wvT_d = nc.dram_tensor("wvT", [DIM, E], F32R, kind="ExternalInput").ap()
    woT_d = nc.dram_tensor("woT", [DIM, DIM], BF16, kind="ExternalInput").ap()
    cos_d = nc.dram_tensor("cosw", [128, S], BF16, kind="ExternalInput").ap()
    sin_d = nc.dram_tensor("sinw", [128, S], BF16, kind="ExternalInput").ap()
    tri_d = nc.dram_tensor("tri", [128, 128], BF16, kind="ExternalInput").ap()
    mz_d = nc.dram_tensor("mz", [128, 256], BF16, kind="ExternalInput").ap()
    lam_d = nc.dram_tensor("lam", [1, 1], F32, kind="ExternalInput").ap()
    out_d = nc.dram_tensor("out", [B, E, DIM], F32, kind="ExternalOutput").ap()

    with tile.TileContext(nc) as tc, \
         nc.allow_low_precision("bf16 attention pipeline by design"):
        for rep in range(nrep):
            ctx = ExitStack()
            consts = ctx.enter_context(tc.tile_pool(name="consts", bufs=1))
            tri_t = consts.tile([128, 128], BF16)
            nc.sync.dma_start(out=tri_t, in_=tri_d)
            mz_t = consts.tile([128, 256], BF16)
            nc.sync.dma_start(out=mz_t, in_=mz_d)
            lam_t = consts.tile([1, 1], F32)
            nc.sync.dma_start(out=lam_t, in_=lam_d)
            ones_col = consts.tile([128, 1], BF16)
            nc.vector.memset(ones_col, 1.0)
            cos_t = consts.tile([128, S], BF16)
            nc.sync.dma_start(out=cos_t, in_=cos_d)
            sin_t = consts.tile([128, S], BF16)
            nc.sync.dma_start(out=sin_t, in_=sin_d)

            # resident roped-projection tiles (bf16)
            qk = ctx.enter_context(tc.tile_pool(name="qk", bufs=1))
            qr, kr = {}, {}
            for hl in range(HPC):
                for b_ in range(B):
                    qr[(hl, b_)] = qk.tile([128, S], BF16, tag=f"qr{hl}{b_}",
                                           name=f"qr{hl}{b_}")
                    kr[(hl, b_)] = qk.tile([128, S], BF16, tag=f"kr{hl}{b_}",
                                           name=f"kr{hl}{b_}")
            v2 = {}
            for b_ in range(B):
                v2[b_] = qk.tile([128, HPC * S], BF16, tag=f"v{b_}",
                                 name=f"v{b_}")
            v2v = {b_: v2[b_][:].rearrange("p (h tt u) -> p h tt u",
                                           h=HPC, tt=S // 128)
                   for b_ in range(B)}
            ropep = ctx.enter_context(tc.tile_pool(name="ropep", bufs=1))

            def rope(b_):
                # q/k rows are host-permuted to [even|odd|even|odd] 32-blocks;
                # partner partitions live in the 32-block xor 1.
                for hl in range(HPC):
                    for t_res in (qr[(hl, b_)], kr[(hl, b_)]):
                        sw = ropep.tile([128, S], BF16, tag="sw", name="sw",
                                        bufs=2)
                        for blk in range(4):
                            sb = (blk ^ 1) * 32
                            nc.sync.dma_start(
                                out=sw[blk * 32:(blk + 1) * 32, :],
                                in_=t_res[sb:sb + 32, :])
                        for w4 in range(4):
                            w = slice(w4 * 512, w4 * 512 + 512)
                            m1 = ropep.tile([128, 512], F32, tag="m1",
                                            name="m1", bufs=4)
                            nc.vector.tensor_mul(m1, t_res[:, w], cos_t[:, w])
                            nc.vector.tensor_mul(sw[:, w], sw[:, w],
                                                 sin_t[:, w])
                            nc.vector.tensor_add(t_res[:, w], m1, sw[:, w])

            # ---------------- phase B: projections ----------------
            bctx = ExitStack()
            wres = bctx.enter_context(tc.tile_pool(name="wres", bufs=1))
            w3 = {}
            for nm, wd in (("q", wqT_d), ("k", wkT_d), ("v", wvT_d)):
                wt = wres.tile([128, ND * E], F32, tag=f"w{nm}", name=f"w{nm}")
                v_ = wt[:].rearrange("p (d e) -> p d e", d=ND)
                nc.sync.dma_start(out=v_.bitcast(F32R),
                                  in_=wd.rearrange("(d p) e -> p d e", p=128))
                w3[nm] = v_
            xs = bctx.enter_context(tc.tile_pool(name="xs", bufs=20))
            psB = bctx.enter_context(tc.tile_pool(name="psB", bufs=1,
                                                  space="PSUM"))
            for tb in range(NTB):
                b_, trel = divmod(tb, 4)
                psq = [psB.tile([128, 512], F32, tag=f"psq{et}",
                                name=f"psq{et}") for et in range(2)]
                psk = [psB.tile([128, 512], F32, tag=f"psk{et}",
                                name=f"psk{et}") for et in range(2)]
                psv = [psB.tile([128, 256], F32, tag=f"psv{ts}",
                                name=f"psv{ts}") for ts in range(4)]
                xts = []
                for dt in range(ND):
                    xt = xs.tile([128, 512], F32, tag="xt", name="xt")
                    nc.sync.dma_start(
                        out=xt.bitcast(F32R),
                        in_=xT_d[dt * 128:(dt + 1) * 128,
                                 tb * 512:(tb + 1) * 512])
                    xts.append(xt)
                    for et in range(2):
                        _mmr(nc, psq[et],
                             w3["q"][:, dt, et * 128:(et + 1) * 128], xt,
                             start=(dt == 0), stop=(dt == ND - 1),
                             skip_group_check=True)
                for dt in range(ND):
                    for et in range(2):
                        _mmr(nc, psk[et],
                             w3["k"][:, dt, et * 128:(et + 1) * 128], xts[dt],
                             start=(dt == 0), stop=(dt == ND - 1),
                             skip_group_check=True)
                for dt in range(ND):
                    for ts in range(4):
                        _mmr(nc, psv[ts],
                             xts[dt][:, ts * 128:(ts + 1) * 128],
                             w3["v"][:, dt, :],
                             start=(dt == 0), stop=(dt == ND - 1),
                             skip_group_check=True)
                for et in range(2):
                    # fold the 1/sqrt(HHD) score scale into Q here
                    nc.scalar.activation(
                        out=qr[(et, b_)][:, trel * 512:(trel + 1) * 512],
                        in_=psq[et], func=AF.Copy, scale=0.125)
                    nc.scalar.copy(
                        out=kr[(et, b_)][:, trel * 512:(trel + 1) * 512],
                        in_=psk[et])
                for ts in range(4):
                    tt = trel * 4 + ts
                    nc.scalar.copy(
                        out=v2v[b_][:, :, tt, :],
                        in_=psv[ts][:].rearrange("p (h u) -> p h u", h=HPC))
                if trel == 3:
                    rope(b_)
            bctx.close()

            # ---------------- results tiles (right side) ----------------
            rctx = ExitStack()
            atp = rctx.enter_context(tc.tile_pool(name="atp", bufs=1,
                                                  side="right"))
            at, msb = {}, {}
            for b_ in range(B):
                for hl in range(HPC):
                    at[(b_, hl)] = atp.tile([128, S], BF16,
                                            tag=f"at{b_}{hl}",
                                            name=f"at{b_}{hl}")
                    msb[(b_, hl)] = atp.tile([128, S], F32,
                                             tag=f"ms{b_}{hl}",
                                             name=f"ms{b_}{hl}")

            # ---------------- phase D: attention ----------------
            dctx = ExitStack()
            expp = dctx.enter_context(tc.tile_pool(name="expp", bufs=3))
            cmb = dctx.enter_context(tc.tile_pool(name="cmb", bufs=1))
            dn = dctx.enter_context(tc.tile_pool(name="dn", bufs=1))
            psS = dctx.enter_context(tc.tile_pool(name="psS", bufs=2,
                                                  space="PSUM"))
            psAU = dctx.enter_context(tc.tile_pool(name="psAU", bufs=2,
                                                   space="PSUM"))
            psL = dctx.enter_context(tc.tile_pool(name="psL", bufs=1,
                                                  space="PSUM"))

            def dnorm(b_, hl):
                # attn *= rsqrt(ms); subw*(1-lambda_init) folded into wo
                at_t, ms_t = at[(b_, hl)], msb[(b_, hl)]
                rinv = dn.tile([128, S], F32, tag="rinv", name="rinv")
                nc.vector.reciprocal_approx_fast(out=rinv, in_=ms_t)
                rs = dn.tile([128, S], F32, tag="rs", name="rs")
                nc.scalar.activation(out=rs, in_=rinv, func=AF.Sqrt)
                nc.vector.tensor_mul(at_t[:], at_t[:], rs)

            for b_ in range(B):
                for hl in range(HPC):
                    at_t, ms_t = at[(b_, hl)], msb[(b_, hl)]
                    q_, k_ = qr[(hl, b_)], kr[(hl, b_)]
                    for ib in range(4):
                        i0 = ib * 512
                        njt = ib * 4 + 4
                        au = [psAU.tile([128, 512], F32, tag=f"au{s_}",
                                        name=f"au{s_}") for s_ in range(2)]
                        Lp = [psL.tile([1, 512], F32, tag=f"L{s_}",
                                       name=f"L{s_}")[:] for s_ in range(2)]
                        for jt in range(njt):
                            j0 = jt * 128
                            r = jt - ib * 4
                            c0 = 0 if r < 0 else min(r * 128, 256)
                            for s_ in range(2):
                                e0 = s_ * 64
                                sp = psS.tile([128, 512], F32, tag="sp",
                                              name="sp")
                                nc.tensor.matmul(
                                    sp[:, c0:512],
                                    k_[e0:e0 + 64, j0:j0 + 128],
                                    q_[e0:e0 + 64, i0 + c0:i0 + 512],
                                    skip_group_check=True)
                                ex = expp.tile([128, 512], BF16,
                                               tag=f"ex{s_}", name=f"ex{s_}")
                                nc.scalar.activation(out=ex[:, c0:512],
                                                     in_=sp[:, c0:512],
                                                     func=AF.Exp)
                                if r >= 0:
                                    if r == 3:
                                        nc.vector.tensor_mul(
                                            ex[:, 256:512], ex[:, 256:512],
                                            mz_t)
                                    else:
                                        m0 = r * 128
                                        nc.vector.tensor_mul(
                                            ex[:, m0:m0 + 128],
                                            ex[:, m0:m0 + 128], tri_t)
                                nc.tensor.matmul(
                                    au[s_][:, c0:512],
                                    v2v[b_][:, hl, jt, :], ex[:, c0:512],
                                    start=(jt == 0), stop=(jt == njt - 1),
                                    skip_group_check=True)
                                nc.tensor.matmul(
                                    Lp[s_][:, c0:512], ones_col,
                                    ex[:, c0:512],
                                    start=(jt == 0), stop=(jt == njt - 1),
                                    skip_group_check=True)
                        # combine: au0/L0 - lam*au1/L1; ms via gpsimd
                        r1 = cmb.tile([1, 512], F32, tag="r1", name="r1")
                        r2 = cmb.tile([1, 512], F32, tag="r2", name="r2")
                        nc.vector.reciprocal_approx_fast(out=r1, in_=Lp[0])
                        nc.vector.reciprocal_approx_fast(out=r2, in_=Lp[1])
                        nc.vector.tensor_scalar_mul(r2, r2, lam_t[:])
                        br1 = cmb.tile([128, 512], F32, tag="br1", name="br1")
                        br2 = cmb.tile([128, 512], F32, tag="br2", name="br2")
                        nc.gpsimd.partition_broadcast(br1, r1)
                        nc.gpsimd.partition_broadcast(br2, r2)
                        t1 = cmb.tile([128, 512], F32, tag="t1", name="t1")
                        t2 = cmb.tile([128, 512], F32, tag="t2", name="t2")
                        nc.vector.tensor_mul(t1, au[0], br1)
                        nc.vector.tensor_mul(t2, au[1], br2)
                        at_s = at_t[:, i0:i0 + 512]
                        nc.vector.tensor_sub(at_s, t1, t2)
                        sq = cmb.tile([128, 512], BF16, tag="sq", name="sq")
                        nc.vector.tensor_mul(sq, at_s, at_s)
                        mstmp = cmb.tile([128, 512], F32, tag="mst",
                                         name="mst", bufs=2)
                        nc.gpsimd.partition_all_reduce(
                            mstmp, sq, channels=128,
                            reduce_op=bass_isa.ReduceOp.add)
                        nc.vector.tensor_scalar(
                            ms_t[:, i0:i0 + 512], mstmp, 1.0 / 128.0, EPS,
                            OP.mult, OP.add)
                    if b_ == 1 and hl == 0:
                        dnorm(0, 0)
                        dnorm(0, 1)
            dnorm(1, 0)
            dnorm(1, 1)
            dctx.close()

            # ---------------- phase C: output projection ----------------
            cctx = ExitStack()
            wop = cctx.enter_context(tc.tile_pool(name="wop", bufs=2,
                                                  side="right"))
            psO = cctx.enter_context(tc.tile_pool(name="psO", bufs=2,
                                                  space="PSUM"))
            ostg = cctx.enter_context(tc.tile_pool(name="ostg", bufs=2))
            for mb in range(4):
                wot = wop.tile([128, 16 * 512], BF16, tag="wot", name="wot")
                nc.sync.dma_start(
                    out=wot[:].rearrange("p (v n) -> p v n", v=16),
                    in_=woT_d[:, mb * 512:(mb + 1) * 512].rearrange(
                        "(v p) n -> p v n", p=128))
                for b_ in range(B):
                    for hl in range(HPC):
                        y = at[(b_, hl)][:].rearrange("p (u v) -> p v u",
                                                      v=16)
                        op = psO.tile([128, 512], F32, tag="op", name="op")
                        for v16 in range(16):
                            nc.tensor.matmul(
                                op, y[:, v16, :],
                                wot[:, v16 * 512:(v16 + 1) * 512],
                                start=(v16 == 0), stop=(v16 == 15),
                                skip_group_check=True)
                        ost = ostg.tile([128, 512], F32, tag="ost",
                                        name="ost")
                        nc.scalar.copy(out=ost, in_=op)
                        nc.sync.dma_start(
                            out=out_d[b_, hl * 128:(hl + 1) * 128,
                                      mb * 512:(mb + 1) * 512],
                            in_=ost)
            cctx.close()
            rctx.close()
            ctx.close()

    nc.compile()
    return nc


def get_program(nrep=1):
    key = f"nc{nrep}"
    if key not in _CACHE:
        _CACHE[key] = _build_program(nrep)
    return _CACHE[key]


def _prep_in_maps(inputs):
    inp = {k: np.ascontiguousarray(np.asarray(v, dtype=np.float32))
           for k, v in inputs.items()}
    # RoPE feature perm per head: [even(0:32) | odd(0:32) | even | odd]
    perm = np.concatenate([
        np.arange(0, 64, 2), np.arange(1, 64, 2),
        np.arange(64, 128, 2), np.arange(65, 128, 2)])
    wq_p = inp["wq"].reshape(NH, HD, DIM)[:, perm, :].reshape(NH * HD, DIM)
    wk_p = inp["wk"].reshape(NH, HD, DIM)[:, perm, :].reshape(NH * HD, DIM)

    fc = inp["freq_cis"]
    cosP = fc[:, :, 0, 0].T.astype(np.float32)
    sinP = fc[:, :, 1, 0].T.astype(np.float32)
    COS = np.concatenate([cosP[0:32], cosP[0:32], cosP[32:64], cosP[32:64]], 0)
    SIN = np.concatenate([-sinP[0:32], sinP[0:32], -sinP[32:64], sinP[32:64]],
                         0)

    tri = (np.arange(128)[:, None] <= np.arange(128)[None, :])
    mz = np.concatenate([np.zeros((128, 128), bool), tri], axis=1)

    lam1 = np.exp(np.sum(inp["lambda_q1"] * inp["lambda_k1"],
                         dtype=np.float32))
    lam2 = np.exp(np.sum(inp["lambda_q2"] * inp["lambda_k2"],
                         dtype=np.float32))
    lam = np.array([[lam1 - lam2 + LAMBDA_INIT]], np.float32)

    # fold subln_w * (1 - lambda_init) into wo^T rows (row f' scales by
    # subw[f' % 128])
    subs = (inp["subln_w"] * (1.0 - LAMBDA_INIT)).astype(np.float32)
    woT = inp["wo"].T * np.tile(subs, NH)[:, None]

    xT = np.ascontiguousarray(inp["x"].reshape(T, DIM).T)
    common = {
        "xT": xT,
        "woT": np.ascontiguousarray(woT.astype(BF)),
        "cosw": np.ascontiguousarray(COS.astype(BF)),
        "sinw": np.ascontiguousarray(SIN.astype(BF)),
        "tri": np.ascontiguousarray(tri.astype(BF)),
        "mz": np.ascontiguousarray(mz.astype(BF)),
        "lam": lam,
    }
    in_maps = []
    for c in range(NC):
        m = dict(common)
        m["wqT"] = np.ascontiguousarray(wq_p[c * E:(c + 1) * E].T)
        m["wkT"] = np.ascontiguousarray(wk_p[c * E:(c + 1) * E].T)
        m["wvT"] = np.ascontiguousarray(inp["wv"][c * E:(c + 1) * E].T)
        in_maps.append(m)
    return in_maps


def run(inputs, trace=False, **kw):
    nc = get_program()
    in_maps = _prep_in_maps(inputs)
    res = run_bass_kernel_spmd(nc, in_maps, list(range(NC)), trace=trace, **kw)
    out = np.empty((B, S, DIM), np.float32)
    for c in range(NC):
        out[:, c * E:(c + 1) * E, :] = res.results[c]["out"]
    return out, res


def kernel(**inputs):
    out, _ = run(inputs)
    return out


# ---------------------------------------------------------------------------
# benchmarking helpers (wall-clock with device-resident inputs, null-calibrated)
# ---------------------------------------------------------------------------

def _make_sharded_callable(nc, in_maps, n_cores):
    import jax
    from jax.experimental.shard_map import shard_map
    from jax.sharding import Mesh, PartitionSpec, NamedSharding
    from concourse import bass2jax

    bass2jax.install_neuronx_cc_hook()
    partition_name = nc.partition_id_tensor.name if nc.partition_id_tensor else None
    in_names, out_names, out_avals, zero_outs = [], [], [], []
    for alloc in nc.m.functions[0].allocations:
        if not isinstance(alloc, mybir.MemoryLocationSet):
            continue
        name = alloc.memorylocations[0].name
        if alloc.kind == "ExternalInput":
            if name != partition_name:
                in_names.append(name)
        elif alloc.kind == "ExternalOutput":
            out_names.append(name)
            shape = tuple(alloc.tensor_shape)
            dtype = mybir.dt.np(alloc.dtype)
            out_avals.append(jax.core.ShapedArray(shape, dtype))
            zero_outs.append(np.zeros(shape, dtype))
    n_params = len(in_names)
    all_in = list(in_names) + list(out_names)
    if partition_name is not None:
        all_in.append(partition_name)

    def _body(*args):
        operands = list(args)
        if partition_name is not None:
            operands.append(bass2jax.partition_id_tensor())
        outs = bass2jax._bass_exec_p.bind(
            *operands,
            out_avals=tuple(out_avals),
            in_names=tuple(all_in),
            out_names=tuple(out_names),
            lowering_input_output_aliases=(),
            sim_require_finite=True,
            sim_require_nnan=True,
            nc=nc,
        )
        return tuple(outs)

    devices = jax.devices()[:n_cores]
    mesh = Mesh(np.asarray(devices), ("core",))
    in_specs = (PartitionSpec("core"),) * (n_params + len(out_names))
    out_specs = (PartitionSpec("core"),) * len(out_names)
    fn = jax.jit(shard_map(_body, mesh=mesh, in_specs=in_specs,
                           out_specs=out_specs, check_rep=False),
                 keep_unused=True)
    sh = NamedSharding(mesh, PartitionSpec("core"))
    per_core = [[np.asarray(m[n]) for n in in_names] for m in in_maps]
    args = [np.concatenate([per_core[c][i] for c in range(n_cores)], axis=0)
            for i in range(n_params)]
    args += [np.zeros((n_cores * z.shape[0], *z.shape[1:]), z.dtype)
             for z in zero_outs]
    dev_args = [jax.device_put(a, sh) for a in args]
    return fn, dev_args


def _time_calls(fn, dev_args, iters=8):
    import time as _t
    import jax
    out = fn(*dev_args)
    jax.block_until_ready(out)
    times = []
    for _ in range(iters):
        t0 = _t.perf_counter()
        out = fn(*dev_args)
        jax.block_until_ready(out)
        times.append(_t.perf_counter() - t0)
    return min(times), times


def _build_null_program():
    nc = bacc.Bacc("TRN2", target_bir_lowering=False, debug=False, num_devices=NC)
    o = nc.dram_tensor("nout", [1, 16], F32, kind="ExternalOutput").ap()
    with tile.TileContext(nc) as tc:
        with tc.tile_pool(name="p", bufs=1) as p:
            t_ = p.tile([1, 16], F32)
            nc.vector.memset(t_, 1.0)
            nc.sync.dma_start(out=o, in_=t_)
    nc.compile()
    return nc


def bench(inputs, iters=8):
    """Returns (kernel_min_s, null_min_s, est_exec_ns)."""
    nc = get_program()
    in_maps = _prep_in_maps(inputs)
    fn, dev_args = _make_sharded_callable(nc, in_maps, NC)
    tk, tk_all = _time_calls(fn, dev_args, iters)
    ncn = _build_null_program()
    fn0, dev0 = _make_sharded_callable(ncn, [{} for _ in range(NC)], NC)
    t0_, t0_all = _time_calls(fn0, dev0, iters)
    return tk, t0_, (tk - t0_) * 1e9, tk_all, t0_all


def bench_rep(inputs, nrep=3, iters=20):
    """(T_nrep - T_1)/(nrep-1) from min wall times; dispatch cancels."""
    in_maps = _prep_in_maps(inputs)
    nc1 = get_program(1)
    fn1, dev1 = _make_sharded_callable(nc1, in_maps, NC)
    t1, t1_all = _time_calls(fn1, dev1, iters)
    ncN = get_program(nrep)
    fnN, devN = _make_sharded_callable(ncN, in_maps, NC)
    tN, tN_all = _time_calls(fnN, devN, iters)
    per = (tN - t1) / (nrep - 1)
    return per, t1, tN, t1_all, tN_all


# revision 21
# speedup vs baseline: 2.6235x; 1.3322x over previous
"""DiffAttention Trainium2 Bass kernel (8-core head-parallel SPMD), v2.

Contract: kernel(**inputs) takes the FULL inputs from setup_inputs() and
returns the FULL (B, S, DIM) output. Internally it shards the 16 heads
across 8 NeuronCores (2 heads/core); each core is fully independent (the
reference's transpose-then-reshape makes each head own a contiguous block
of 256 output rows, so no collectives are needed).

v2 design (vs the DRAM-spill baseline):
  - host pre-transposes x, wq/wk/wv (with the RoPE even/odd row perm) and
    wo, so the device does ZERO PE transposes; subln_w and (1-lambda_init)
    are folded into wo on the host; the 1/sqrt(64) score scale is folded
    into the Q PSUM eviction.
  - phase B computes Q^T/K^T (feature-major) and V (token-major) straight
    into resident SBUF tiles (bf16) -- no DRAM spill/reload.
  - RoPE per (head, batch) right after its 4 token-blocks finish: SBUF->
    SBUF partition-swap DMA + 3 DVE ops against host-baked cos/sin tables.
  - flash-style two-stream attention on transposed scores (keys on
    partitions): fp32r scores matmul, scalar-engine exp -> bf16, bf16 PV
    and ones-row-sum matmuls; diagonal tiles clamp the column start to 256
    so every matmul keeps the fp32r full-rate (>=256 moving) shape.
  - mean-square for the RMSNorm via gpsimd partition_all_reduce (keeps all
    8 PSUM banks for scores/PV/L); deferred norm uses DVE
    reciprocal_approx_fast + batched scalar Sqrt (avoids act-table thrash).
  - output projection with bf16 wo^T tiles, "scrambled reshape" expressed
    as a stride-16 stationary AP.
"""

import numpy as np
from contextlib import ExitStack

import ml_dtypes

import concourse.bass as bass
import concourse.bacc as bacc
import concourse.tile as tile
from concourse import mybir, bass_isa
from concourse.bass_utils import run_bass_kernel_spmd

F32 = mybir.dt.float32
F32R = mybir.dt.float32r
BF16 = mybir.dt.bfloat16
AF = mybir.ActivationFunctionType
OP = mybir.AluOpType

B, S, DIM = 2, 2048, 2048
NH, HD, HHD = 16, 128, 64
NC = 8
HPC = NH // NC          # 2 heads per core
E = HPC * HD            # 256 projection rows per core
T = B * S               # 4096 flattened tokens
ND = DIM // 128         # 16 d-tiles
NTB = T // 512          # 8 t-blocks
LAMBDA_INIT = 0.2
EPS = 1e-5

BF = ml_dtypes.bfloat16


def _mmr(nc, out, lhsT, rhs, **kw):
    """matmul with float32r operand views (full PE rate at >=256 moving)."""
    nc.tensor.matmul(out, lhsT.bitcast(F32R), rhs.bitcast(F32R), **kw)


_CACHE = {}


def _build_program(nrep=1):
    nc = bacc.Bacc("TRN2", target_bir_lowering=False, debug=False, num_devices=NC)

    xT_d = nc.dram_tensor("xT", [DIM, T], F32R, kind="ExternalInput").ap()
    wqT_d = nc.dram_tensor("wqT", [DIM, E], F32R, kind="ExternalInput").ap()
    wkT_d = nc.dram_tensor("wkT", [DIM, E], F32R, kind="ExternalInput").ap()
    wvT_d = nc.dram_tensor("wvT", [DIM, E], F32R, kind="ExternalInput").ap()
    woT_d = nc.dram_tensor("woT", [DIM, DIM], BF16, kind="ExternalInput").ap()
    cos_d = nc.dram_tensor("cosw", [128, S], BF16, kind="ExternalInput").ap()
    sin_d = nc.dram_tensor("sinw", [128, S], BF16, kind="ExternalInput").ap()
    tri_d = nc.dram_tensor("tri", [128, 128], BF16, kind="ExternalInput").ap()
    mz_d = nc.dram_tensor("mz", [128, 256], BF16, kind="ExternalInput").ap()
    lam_d = nc.dram_tensor("lam", [1, 1], F32, kind="ExternalInput").ap()
    out_d = nc.dram_tensor("out", [B, E, DIM], F32, kind="ExternalOutput").ap()

    with tile.TileContext(nc) as tc, \
         nc.allow_low_precision("bf16 attention pipeline by design"):
        for rep in range(nrep):
            ctx = ExitStack()
            consts = ctx.enter_context(tc.tile_pool(name="consts", bufs=1))
            tri_t = consts.tile([128, 128], BF16)
            nc.sync.dma_start(out=tri_t, in_=tri_d)
            mz_t = consts.tile([128, 256], BF16)
            nc.sync.dma_start(out=mz_t, in_=mz_d)
            lam_t = consts.tile([1, 1], F32)
            nc.sync.dma_start(out=lam_t, in_=lam_d)
            ones_col = consts.tile([128, 1], BF16)
            nc.vector.memset(ones_col, 1.0)
            cos_t = consts.tile([128, S], BF16)
            nc.sync.dma_start(out=cos_t, in_=cos_d)
            sin_t = consts.tile([128, S], BF16)
            nc.sync.dma_start(out=sin_t, in_=sin_d)

            # resident roped-projection tiles (bf16)
            qk = ctx.enter_context(tc.tile_pool(name="qk", bufs=1))
            qr, kr = {}, {}
            for hl in range(HPC):
                for b_ in range(B):
                    qr[(hl, b_)] = qk.tile([128, S], BF16, tag=f"qr{hl}{b_}",
                                           name=f"qr{hl}{b_}")
                    kr[(hl, b_)] = qk.tile([128, S], BF16, tag=f"kr{hl}{b_}",
                                           name=f"kr{hl}{b_}")
            v2 = {}
            for b_ in range(B):
                v2[b_] = qk.tile([128, HPC * S], BF16, tag=f"v{b_}",
                                 name=f"v{b_}")
            v2v = {b_: v2[b_][:].rearrange("p (h tt u) -> p h tt u",
                                           h=HPC, tt=S // 128)
                   for b_ in range(B)}
            ropep = ctx.enter_context(tc.tile_pool(name="ropep", bufs=1))

            def rope(b_):
                # q/k rows are host-permuted to [even|odd|even|odd] 32-blocks;
                # partner partitions live in the 32-block xor 1.
                for hl in range(HPC):
                    for t_res in (qr[(hl, b_)], kr[(hl, b_)]):
                        sw = ropep.tile([128, S], BF16, tag="sw", name="sw",
                                        bufs=2)
                        for blk in range(4):
                            sb = (blk ^ 1) * 32
                            nc.sync.dma_start(
                                out=sw[blk * 32:(blk + 1) * 32, :],
                                in_=t_res[sb:sb + 32, :])
                        for w4 in range(4):
                            w = slice(w4 * 512, w4 * 512 + 512)
                            m1 = ropep.tile([128, 512], F32, tag="m1",
                                            name="m1", bufs=4)
                            nc.vector.tensor_mul(m1, t_res[:, w], cos_t[:, w])
                            nc.vector.tensor_mul(sw[:, w], sw[:, w],
                                                 sin_t[:, w])
                            nc.vector.tensor_add(t_res[:, w], m1, sw[:, w])

            # ---------------- phase B: projections ----------------
            bctx = ExitStack()
            wres = bctx.enter_context(tc.tile_pool(name="wres", bufs=1))
            w3 = {}
            for nm, wd in (("q", wqT_d), ("k", wkT_d), ("v", wvT_d)):
                wt = wres.tile([128, ND * E], F32, tag=f"w{nm}", name=f"w{nm}")
                v_ = wt[:].rearrange("p (d e) -> p d e", d=ND)
                nc.sync.dma_start(out=v_.bitcast(F32R),
                                  in_=wd.rearrange("(d p) e -> p d e", p=128))
                w3[nm] = v_
            xs = bctx.enter_context(tc.tile_pool(name="xs", bufs=20))
            psB = bctx.enter_context(tc.tile_pool(name="psB", bufs=1,
                                                  space="PSUM"))
            for tb in range(NTB):
                b_, trel = divmod(tb, 4)
                psq = [psB.tile([128, 512], F32, tag=f"psq{et}",
                                name=f"psq{et}") for et in range(2)]
                psk = [psB.tile([128, 512], F32, tag=f"psk{et}",
                                name=f"psk{et}") for et in range(2)]
                psv = [psB.tile([128, 256], F32, tag=f"psv{ts}",
                                name=f"psv{ts}") for ts in range(4)]
                xts = []
                for dt in range(ND):
                    xt = xs.tile([128, 512], F32, tag="xt", name="xt")
                    nc.sync.dma_start(
                        out=xt.bitcast(F32R),
                        in_=xT_d[dt * 128:(dt + 1) * 128,
                                 tb * 512:(tb + 1) * 512])
                    xts.append(xt)
                    for et in range(2):
                        _mmr(nc, psq[et],
                             w3["q"][:, dt, et * 128:(et + 1) * 128], xt,
                             start=(dt == 0), stop=(dt == ND - 1),
                             skip_group_check=True)
                for dt in range(ND):
                    for et in range(2):
                        _mmr(nc, psk[et],
                             w3["k"][:, dt, et * 128:(et + 1) * 128], xts[dt],
                             start=(dt == 0), stop=(dt == ND - 1),
                             skip_group_check=True)
                for dt in range(ND):
                    for ts in range(4):
                        _mmr(nc, psv[ts],
                             xts[dt][:, ts * 128:(ts + 1) * 128],
                             w3["v"][:, dt, :],
                             start=(dt == 0), stop=(dt == ND - 1),
                             skip_group_check=True)
                for et in range(2):
                    # fold the 1/sqrt(HHD) score scale into Q here
                    nc.scalar.activation(
                        out=qr[(et, b_)][:, trel * 512:(trel + 1) * 512],
                        in_=psq[et], func=AF.Copy, scale=0.125)
                    nc.scalar.copy(
                        out=kr[(et, b_)][:, trel * 512:(trel + 1) * 512],
                        in_=psk[et])
                for ts in range(4):
                    tt = trel * 4 + ts
                    nc.scalar.copy(
                        out=v2v[b_][:, :, tt, :],
                        in_=psv[ts][:].rearrange("p (h u) -> p h u", h=HPC))
                if trel == 3:
                    rope(b_)
            bctx.close()

            # ---------------- results tiles (right side) ----------------
            rctx = ExitStack()
            atp = rctx.enter_context(tc.tile_pool(name="atp", bufs=1,
                                                  side="right"))
            at, msb = {}, {}
            for b_ in range(B):
                for hl in range(HPC):
                    at[(b_, hl)] = atp.tile([128, S], BF16,
                                            tag=f"at{b_}{hl}",
                                            name=f"at{b_}{hl}")
                    msb[(b_, hl)] = atp.tile([128, S], F32,
                                             tag=f"ms{b_}{hl}",
                                             name=f"ms{b_}{hl}")

            # ---------------- phase D: attention ----------------
            dctx = ExitStack()
            expp = dctx.enter_context(tc.tile_pool(name="expp", bufs=3))
            cmb = dctx.enter_context(tc.tile_pool(name="cmb", bufs=1))
            dn = dctx.enter_context(tc.tile_pool(name="dn", bufs=1))
            psS = dctx.enter_context(tc.tile_pool(name="psS", bufs=2,
                                                  space="PSUM"))
            psAU = dctx.enter_context(tc.tile_pool(name="psAU", bufs=2,
                                                   space="PSUM"))
            psL = dctx.enter_context(tc.tile_pool(name="psL", bufs=1,
                                                  space="PSUM"))

            def dnorm(b_, hl):
                # attn *= rsqrt(ms); subw*(1-lambda_init) folded into wo
                at_t, ms_t = at[(b_, hl)], msb[(b_, hl)]
                rinv = dn.tile([128, S], F32, tag="rinv", name="rinv")
                nc.vector.reciprocal_approx_fast(out=rinv, in_=ms_t)
                rs = dn.tile([128, S], F32, tag="rs", name="rs")
                nc.scalar.activation(out=rs, in_=rinv, func=AF.Sqrt)
                nc.vector.tensor_mul(at_t[:], at_t[:], rs)

            for b_ in range(B):
                for hl in range(HPC):
                    at_t, ms_t = at[(b_, hl)], msb[(b_, hl)]
                    q_, k_ = qr[(hl, b_)], kr[(hl, b_)]
                    for ib in range(4):
                        i0 = ib * 512
                        njt = ib * 4 + 4
                        au = [psAU.tile([128, 512], F32, tag=f"au{s_}",
                                        name=f"au{s_}") for s_ in range(2)]
                        Lp = [psL.tile([1, 512], F32, tag=f"L{s_}",
                                       name=f"L{s_}")[:] for s_ in range(2)]
                        for jt in range(njt):
                            j0 = jt * 128
                            r = jt - ib * 4
                            # clamp the diagonal start to 256: matmuls below
                            # ~256 moving rows are latency-bound and break
                            # the exp->PV pipelining (measured regression)
                            c0 = 0 if r < 0 else min(r * 128, 256)
                            for s_ in range(2):
                                e0 = s_ * 64
                                sp = psS.tile([128, 512], F32, tag="sp",
                                              name="sp")
                                nc.tensor.matmul(
                                    sp[:, c0:512],
                                    k_[e0:e0 + 64, j0:j0 + 128],
                                    q_[e0:e0 + 64, i0 + c0:i0 + 512],
                                    skip_group_check=True)
                                ex = expp.tile([128, 512], BF16,
                                               tag=f"ex{s_}", name=f"ex{s_}")
                                nc.scalar.activation(out=ex[:, c0:512],
                                                     in_=sp[:, c0:512],
                                                     func=AF.Exp)
                                if r >= 0:
                                    if r == 3:
                                        nc.vector.tensor_mul(
                                            ex[:, 256:512], ex[:, 256:512],
                                            mz_t)
                                    else:
                                        m0 = r * 128
                                        nc.vector.tensor_mul(
                                            ex[:, m0:m0 + 128],
                                            ex[:, m0:m0 + 128], tri_t)
                                nc.tensor.matmul(
                                    au[s_][:, c0:512],
                                    v2v[b_][:, hl, jt, :], ex[:, c0:512],
                                    start=(jt == 0), stop=(jt == njt - 1),
                                    skip_group_check=True)
                                nc.tensor.matmul(
                                    Lp[s_][:, c0:512], ones_col,
                                    ex[:, c0:512],
                                    start=(jt == 0), stop=(jt == njt - 1),
                                    skip_group_check=True)
                        # combine: au0/L0 - lam*au1/L1; ms via gpsimd
                        r1 = cmb.tile([1, 512], F32, tag="r1", name="r1")
                        r2 = cmb.tile([1, 512], F32, tag="r2", name="r2")
                        nc.vector.reciprocal_approx_fast(out=r1, in_=Lp[0])
                        nc.vector.reciprocal_approx_fast(out=r2, in_=Lp[1])
                        nc.vector.tensor_scalar_mul(r2, r2, lam_t[:])
                        br1 = cmb.tile([128, 512], F32, tag="br1", name="br1")
                        br2 = cmb.tile([128, 512], F32, tag="br2", name="br2")
                        nc.gpsimd.partition_broadcast(br1, r1)
                        nc.gpsimd.partition_broadcast(br2, r2)
                        t1 = cmb.tile([128, 512], F32, tag="t1", name="t1")
                        t2 = cmb.tile([128, 512], F32, tag="t2", name="t2")
                        nc.vector.tensor_mul(t1, au[0], br1)
                        nc.vector.tensor_mul(t2, au[1], br2)
                        at_s = at_t[:, i0:i0 + 512]
                        nc.vector.tensor_sub(at_s, t1, t2)
                        sq = cmb.tile([128, 512], BF16, tag="sq", name="sq")
                        nc.vector.tensor_mul(sq, at_s, at_s)
                        mstmp = cmb.tile([128, 512], F32, tag="mst",
                                         name="mst", bufs=2)
                        nc.gpsimd.partition_all_reduce(
                            mstmp, sq, channels=128,
                            reduce_op=bass_isa.ReduceOp.add)
                        nc.vector.tensor_scalar(
                            ms_t[:, i0:i0 + 512], mstmp, 1.0 / 128.0, EPS,
                            OP.mult, OP.add)
                    if b_ == 1 and hl == 0:
                        dnorm(0, 0)
                        dnorm(0, 1)
            dnorm(1, 0)
            dnorm(1, 1)
            dctx.close()

            # ---------------- phase C: output projection ----------------
            cctx = ExitStack()
            wop = cctx.enter_context(tc.tile_pool(name="wop", bufs=2,
                                                  side="right"))
            psO = cctx.enter_context(tc.tile_pool(name="psO", bufs=2,
                                                  space="PSUM"))
            ostg = cctx.enter_context(tc.tile_pool(name="ostg", bufs=2))
            for mb in range(4):
                wot = wop.tile([128, 16 * 512], BF16, tag="wot", name="wot")
                nc.sync.dma_start(
                    out=wot[:].rearrange("p (v n) -> p v n", v=16),
                    in_=woT_d[:, mb * 512:(mb + 1) * 512].rearrange(
                        "(v p) n -> p v n", p=128))
                for b_ in range(B):
                    for hl in range(HPC):
                        y = at[(b_, hl)][:].rearrange("p (u v) -> p v u",
                                                      v=16)
                        op = psO.tile([128, 512], F32, tag="op", name="op")
                        for v16 in range(16):
                            nc.tensor.matmul(
                                op, y[:, v16, :],
                                wot[:, v16 * 512:(v16 + 1) * 512],
                                start=(v16 == 0), stop=(v16 == 15),
                                skip_group_check=True)
                        ost = ostg.tile([128, 512], F32, tag="ost",
                                        name="ost")
                        nc.scalar.copy(out=ost, in_=op)
                        nc.sync.dma_start(
                            out=out_d[b_, hl * 128:(hl + 1) * 128,
                                      mb * 512:(mb + 1) * 512],
                            in_=ost)
            cctx.close()
            rctx.close()
            ctx.close()

    nc.compile()
    return nc


def get_program(nrep=1):
    key = f"nc{nrep}"
    if key not in _CACHE:
        _CACHE[key] = _build_program(nrep)
    return _CACHE[key]


def _prep_in_maps(inputs):
    inp = {k: np.ascontiguousarray(np.asarray(v, dtype=np.float32))
           for k, v in inputs.items()}
    # RoPE feature perm per head: [even(0:32) | odd(0:32) | even | odd]
    perm = np.concatenate([
        np.arange(0, 64, 2), np.arange(1, 64, 2),
        np.arange(64, 128, 2), np.arange(65, 128, 2)])
    wq_p = inp["wq"].reshape(NH, HD, DIM)[:, perm, :].reshape(NH * HD, DIM)
    wk_p = inp["wk"].reshape(NH, HD, DIM)[:, perm, :].reshape(NH * HD, DIM)

    fc = inp["freq_cis"]
    cosP = fc[:, :, 0, 0].T.astype(np.float32)
    sinP = fc[:, :, 1, 0].T.astype(np.float32)
    COS = np.concatenate([cosP[0:32], cosP[0:32], cosP[32:64], cosP[32:64]], 0)
    SIN = np.concatenate([-sinP[0:32], sinP[0:32], -sinP[32:64], sinP[32:64]],
                         0)

    tri = (np.arange(128)[:, None] <= np.arange(128)[None, :])
    mz = np.concatenate([np.zeros((128, 128), bool), tri], axis=1)

    lam1 = np.exp(np.sum(inp["lambda_q1"] * inp["lambda_k1"],
                         dtype=np.float32))
    lam2 = np.exp(np.sum(inp["lambda_q2"] * inp["lambda_k2"],
                         dtype=np.float32))
    lam = np.array([[lam1 - lam2 + LAMBDA_INIT]], np.float32)

    # fold subln_w * (1 - lambda_init) into wo^T rows (row f' scales by
    # subw[f' % 128])
    subs = (inp["subln_w"] * (1.0 - LAMBDA_INIT)).astype(np.float32)
    woT = inp["wo"].T * np.tile(subs, NH)[:, None]

    xT = np.ascontiguousarray(inp["x"].reshape(T, DIM).T)
    common = {
        "xT": xT,
        "woT": np.ascontiguousarray(woT.astype(BF)),
        "cosw": np.ascontiguousarray(COS.astype(BF)),
        "sinw": np.ascontiguousarray(SIN.astype(BF)),
        "tri": np.ascontiguousarray(tri.astype(BF)),
        "mz": np.ascontiguousarray(mz.astype(BF)),
        "lam": lam,
    }
    in_maps = []
    for c in range(NC):
        m = dict(common)
        m["wqT"] = np.ascontiguousarray(wq_p[c * E:(c + 1) * E].T)
        m["wkT"] = np.ascontiguousarray(wk_p[c * E:(c + 1) * E].T)
        m["wvT"] = np.ascontiguousarray(inp["wv"][c * E:(c + 1) * E].T)
        in_maps.append(m)
    return in_maps


def run(inputs, trace=False, **kw):
    nc = get_program()
    in_maps = _prep_in_maps(inputs)
    res = run_bass_kernel_spmd(nc, in_maps, list(range(NC)), trace=trace, **kw)
    out = np.empty((B, S, DIM), np.float32)
    for c in range(NC):
        out[:, c * E:(c + 1) * E, :] = res.results[c]["out"]
    return out, res


def kernel(**inputs):
    out, _ = run(inputs)
    return out


# ---------------------------------------------------------------------------
# benchmarking helpers (wall-clock with device-resident inputs, null-calibrated)
# ---------------------------------------------------------------------------

def _make_sharded_callable(nc, in_maps, n_cores):
    import jax
    from jax.experimental.shard_map import shard_map
    from jax.sharding import Mesh, PartitionSpec, NamedSharding
    from concourse import bass2jax

    bass2jax.install_neuronx_cc_hook()
    partition_name = nc.partition_id_tensor.name if nc.partition_id_tensor else None
    in_names, out_names, out_avals, zero_outs = [], [], [], []
    for alloc in nc.m.functions[0].allocations:
        if not isinstance(alloc, mybir.MemoryLocationSet):
            continue
        name = alloc.memorylocations[0].name
        if alloc.kind == "ExternalInput":
            if name != partition_name:
                in_names.append(name)
        elif alloc.kind == "ExternalOutput":
            out_names.append(name)
            shape = tuple(alloc.tensor_shape)
            dtype = mybir.dt.np(alloc.dtype)
            out_avals.append(jax.core.ShapedArray(shape, dtype))
            zero_outs.append(np.zeros(shape, dtype))
    n_params = len(in_names)
    all_in = list(in_names) + list(out_names)
    if partition_name is not None:
        all_in.append(partition_name)

    def _body(*args):
        operands = list(args)
        if partition_name is not None:
            operands.append(bass2jax.partition_id_tensor())
        outs = bass2jax._bass_exec_p.bind(
            *operands,
            out_avals=tuple(out_avals),
            in_names=tuple(all_in),
            out_names=tuple(out_names),
            lowering_input_output_aliases=(),
            sim_require_finite=True,
            sim_require_nnan=True,
            nc=nc,
        )
        return tuple(outs)

    devices = jax.devices()[:n_cores]
    mesh = Mesh(np.asarray(devices), ("core",))
    in_specs = (PartitionSpec("core"),) * (n_params + len(out_names))
    out_specs = (PartitionSpec("core"),) * len(out_names)
    fn = jax.jit(shard_map(_body, mesh=mesh, in_specs=in_specs,
                           out_specs=out_specs, check_rep=False),
                 keep_unused=True)
    sh = NamedSharding(mesh, PartitionSpec("core"))
    per_core = [[np.asarray(m[n]) for n in in_names] for m in in_maps]
    args = [np.concatenate([per_core[c][i] for c in range(n_cores)], axis=0)
            for i in range(n_params)]
    args += [np.zeros((n_cores * z.shape[0], *z.shape[1:]), z.dtype)
             for z in zero_outs]
    dev_args = [jax.device_put(a, sh) for a in args]
    return fn, dev_args


def _time_calls(fn, dev_args, iters=8):
    import time as _t
    import jax
    out = fn(*dev_args)
    jax.block_until_ready(out)
    times = []
    for _ in range(iters):
        t0 = _t.perf_counter()
        out = fn(*dev_args)
        jax.block_until_ready(out)
        times.append(_t.perf_counter() - t0)
    return min(times), times


def _build_null_program():
    nc = bacc.Bacc("TRN2", target_bir_lowering=False, debug=False, num_devices=NC)
    o = nc.dram_tensor("nout", [1, 16], F32, kind="ExternalOutput").ap()
    with tile.TileContext(nc) as tc:
        with tc.tile_pool(name="p", bufs=1) as p:
            t_ = p.tile([1, 16], F32)
            nc.vector.memset(t_, 1.0)
            nc.sync.dma_start(out=o, in_=t_)
    nc.compile()
    return nc


def bench(inputs, iters=8):
    """Returns (kernel_min_s, null_min_s, est_exec_ns)."""
    nc = get_program()
    in_maps = _prep_in_maps(inputs)
    fn, dev_args = _make_sharded_callable(nc, in_maps, NC)
    tk, tk_all = _time_calls(fn, dev_args, iters)
    ncn = _build_null_program()
    fn0, dev0 = _make_sharded_callable(ncn, [{} for _ in range(NC)], NC)
    t0_, t0_all = _time_calls(fn0, dev0, iters)
    return tk, t0_, (tk - t0_) * 1e9, tk_all, t0_all


def bench_rep(inputs, nrep=3, iters=20):
    """(T_nrep - T_1)/(nrep-1); t1/tN calls are interleaved and differenced
    pairwise so slow wall-clock drift cancels; median of pairwise deltas."""
    import time as _t
    import jax
    in_maps = _prep_in_maps(inputs)
    nc1 = get_program(1)
    fn1, dev1 = _make_sharded_callable(nc1, in_maps, NC)
    ncN = get_program(nrep)
    fnN, devN = _make_sharded_callable(ncN, in_maps, NC)
    jax.block_until_ready(fn1(*dev1))
    jax.block_until_ready(fnN(*devN))
    t1_all, tN_all = [], []
    for _ in range(iters):
        t0 = _t.perf_counter()
        jax.block_until_ready(fn1(*dev1))
        t1_all.append(_t.perf_counter() - t0)
        t0 = _t.perf_counter()
        jax.block_until_ready(fnN(*devN))
        tN_all.append(_t.perf_counter() - t0)
    deltas = sorted((b - a) / (nrep - 1) for a, b in zip(t1_all, tN_all))
    per = deltas[len(deltas) // 2]
    return per, min(t1_all), min(tN_all), t1_all, tN_all
